# revision 7
# baseline (speedup 1.0000x reference)
"""8-layer GCN on 8 Trainium2 NeuronCores — v2 (fp8 DoubleRow aggregation).

Per conv l: x_l = g_l @ W_l ; t = P @ x_l ; g_{l+1} = relu(dinv^2 t + dinv b).

  L0-L2 (B-form): aggregate the gathered activations first (u = P @ q(g)),
      then transform only the own 1280 rows (px = u16 @ W).  The aggregation
      runs FLIPPED on the tensor engine (stationary = activation fp8 pairs,
      moving = P block pairs) in fp8 DoubleRow perf mode: K=256 per matmul at
      0.5 cycles/col — 2x the fp16 MAC rate.  P entries (small ints) are
      exact in fp8e4.  L0 activations ship as e4m3 hi + e5m2 residual planes,
      both accumulated into the SAME psum bank (the e5m2 residual needs no
      scale); L1/L2 ship the e4m3 hi plane only (validated ~3e-3 final).
  L3-L4 (C-form): transform own rows first, ship x_l as hi|lo fp8 fused,
      flipped DoubleRow aggregation, feature-major epilogue via
      host-precomputed broadcast dinv tiles.
  L5-L7: narrow (w<=64) layers where DoubleRow loses on LDWEIGHTS — plain
      fp16-moving direct aggregation, with the 10 per-j-tile accumulators
      packed into two PSUM banks so the src loop can run chunk-major.

  One DR matmul covers a whole 4-j-tile PSUM bank (moving = [128, 2, 512]
  P-pair group), so the hot loop is 3 matmuls per (src-pair, plane, f-slice).
  Only the first matmul into each bank uses start=True (hw clears the whole
  bank); later slices start=False and overwrite-where-unset.

  Activations ship 1B/value (fp8) node-major in three j-chunks (4/4/2)
  per boundary, each staged as soon as its quarter of the producer work is
  done; the consumer aggregation reads src pairs chunk-major so it starts
  after the first chunk lands.  Gathered buffers ping-pong between two
  20KB/partition SBUF slots (bitcast-viewed per boundary) next to the
  100KB/partition adjacency.
"""

import numpy as np
import ml_dtypes

import concourse.mybir as mybir
import concourse.tile as tile
from concourse import bacc
from concourse import bass_utils
from concourse.masks import make_identity

F16 = np.float16
E4 = ml_dtypes.float8_e4m3
E5 = ml_dtypes.float8_e5m2

N_NODES = 10000
N_CORES = 8
DIMS = [128, 256, 256, 256, 128, 128, 64, 64, 32]
D_OUT = 1
SHARD = N_NODES // N_CORES          # 1250
SHARD_PAD = 1280
NPAD = N_CORES * SHARD_PAD          # 10240
JT = SHARD_PAD // 128               # 10
NB = NPAD // 128                    # 80 src blocks
PAIRS = NB // 2                     # 40

JGROUPS = ((0, 1, 2, 3), (4, 5, 6, 7), (8, 9))   # AG-chunk / psum-bank groups
CH_BLK = [N_CORES * len(js) for js in JGROUPS]    # blocks per chunk: 32,32,16
CH_OFF = [0, 32, 64]                              # block-position offsets

# per-boundary AllGather payload: (width-elems, dtype) node-major rows
AG_CFG = [
    (256, "e4"),   # b0: g1 hi            (input of L1)
    (256, "e4"),   # b1: g2 hi            (input of L2)
    (256, "e4"),   # b2: x3 hi|lo fused   (input of L3)
    (256, "e4"),   # b3: x4 hi|lo fused   (input of L4)
    (64, "f16"),   # b4: x5 fp16          (input of L5)
    (64, "f16"),   # b5: x6 fp16          (input of L6)
    (32, "f16"),   # b6: x7 fp16          (input of L7)
]


def _sigma():
    order = []
    for js in JGROUPS:
        for r in range(N_CORES):
            for j in js:
                order.append(r * JT + j)
    return np.asarray(order, np.int64)


def _build_bass(sim_mode=False, repeats=1):
    nc = bacc.Bacc(
        "TRN2",
        target_bir_lowering=False,
        debug=False,
        enable_asserts=False,
        num_devices=1 if sim_mode else N_CORES,
    )
    dt = mybir.dt
    AF = mybir.ActivationFunctionType
    OP = mybir.AluOpType
    DRM = mybir.MatmulPerfMode.DoubleRow

    # [p, itp, pair, jt, c]
    pt_in = nc.dram_tensor("pt_in", [128, PAIRS, 2, JT, 128], dt.float8e4, kind="ExternalInput").ap()
    g0q_in = nc.dram_tensor("g0q_in", [128, NB, 256], dt.float8e4, kind="ExternalInput").ap()
    d1_in = nc.dram_tensor("d1_in", [128, JT], dt.float32, kind="ExternalInput").ap()
    d2_in = nc.dram_tensor("d2_in", [128, JT], dt.float32, kind="ExternalInput").ap()
    d1bc_in = nc.dram_tensor("d1bc_in", [128, JT, 128], dt.float32, kind="ExternalInput").ap()
    d2bc_in = nc.dram_tensor("d2bc_in", [128, JT, 128], dt.float32, kind="ExternalInput").ap()
    FT = [max(1, DIMS[i] // 128) for i in range(8)]
    w_ins = [
        nc.dram_tensor(f"w{i}_in", [128, FT[i], DIMS[i + 1]], dt.float16, kind="ExternalInput").ap()
        for i in range(8)
    ]
    bb_ins = [
        nc.dram_tensor(f"bb{i}_in", [128, DIMS[i + 1]], dt.float32, kind="ExternalInput").ap()
        for i in range(8)
    ]
    bcol_ins = {
        i: nc.dram_tensor(f"bcol{i}_in", [128, 1], dt.float32, kind="ExternalInput").ap()
        for i in (3, 4)
    }
    wr_in = nc.dram_tensor("wr_in", [128, DIMS[-1]], dt.float32, kind="ExternalInput").ap()
    br_in = nc.dram_tensor("br_in", [128, 1], dt.float32, kind="ExternalInput").ap()
    out_dram = nc.dram_tensor("out", [JT, 128, 1], dt.float32, kind="ExternalOutput").ap()

    ag_in, ag_out = [], []
    for b, (w, dty) in enumerate(AG_CFG):
        d = dt.float8e4 if dty == "e4" else dt.float16
        ag_in.append([
            nc.dram_tensor(f"agi{b}_{ch}", [len(js) * 128, w], d, kind="Internal").ap()
            for ch, js in enumerate(JGROUPS)
        ])
        ag_out.append([
            nc.dram_tensor(
                f"ago{b}_{ch}", [N_CORES * len(js) * 128, w], d,
                kind="Internal", addr_space="Shared",
            ).ap()
            for ch, js in enumerate(JGROUPS)
        ])

    def do_ag(b, ch):
        if sim_mode:
            # model only the core's own local DMA share of the collective
            # (real AllGather traffic rides the NeuronLink DGEs, not the
            # SBUF DMA queues): one own-slot copy; the gathered readback
            # below carries the recv-side volume.
            rows = len(JGROUPS[ch]) * 128
            for r in range(1):
                nc.sync.dma_start(ag_out[b][ch][r * rows:(r + 1) * rows, :], ag_in[b][ch][:])
        else:
            nc.gpsimd.collective_compute(
                "AllGather", mybir.AluOpType.bypass,
                replica_groups=[list(range(N_CORES))],
                ins=[ag_in[b][ch][:]], outs=[ag_out[b][ch][:]],
            )

    with tile.TileContext(nc) as tc:
        with (
            tc.tile_pool(name="const", bufs=1) as const,
            tc.tile_pool(name="work", bufs=1) as work,
            tc.tile_pool(name="upool", bufs=1, space="PSUM") as upool,
            tc.tile_pool(name="wpool", bufs=3, space="PSUM") as wpool,
        ):
            pt_sb = const.tile([128, PAIRS, 2, JT, 128], dt.float8e4)
            # two gathered-activation slots, ping-ponged across boundaries:
            # S0: g0q, g2, x4, x6   S1: g1, x3, x5, x7
            S0 = const.tile([128, NB, 256], dt.float8e4)
            S1 = const.tile([128, NB, 256], dt.float8e4)
            # load order: first agg's chunk-0 data first, then pt slabs
            nc.scalar.dma_start(S0[:, 0:CH_BLK[0], :], g0q_in[:, 0:CH_BLK[0], :])
            for itp in range(4):
                nc.sync.dma_start(pt_sb[:, itp, :, :, :], pt_in[:, itp, :, :, :])
            for ch in range(1, 3):
                q0, q1 = CH_OFF[ch], CH_OFF[ch] + CH_BLK[ch]
                nc.scalar.dma_start(S0[:, q0:q1, :], g0q_in[:, q0:q1, :])
            for itp in range(4, PAIRS):
                nc.sync.dma_start(pt_sb[:, itp, :, :, :], pt_in[:, itp, :, :, :])
            d1 = const.tile([128, JT], dt.float32)
            nc.sync.dma_start(d1[:], d1_in[:])
            d2 = const.tile([128, JT], dt.float32)
            nc.sync.dma_start(d2[:], d2_in[:])
            d1bc = const.tile([128, JT, 128], dt.float32)
            nc.sync.dma_start(d1bc[:], d1bc_in[:])
            d2bc = const.tile([128, JT, 128], dt.float32)
            nc.sync.dma_start(d2bc[:], d2bc_in[:])
            w_sb, bb_sb = [], []
            for li in range(8):
                w = const.tile([128, FT[li], DIMS[li + 1]], dt.float16, name=f"w{li}")
                nc.sync.dma_start(w[:], w_ins[li][:])
                w_sb.append(w)
                bb = const.tile([128, DIMS[li + 1]], dt.float32, name=f"bb{li}")
                nc.sync.dma_start(bb[:], bb_ins[li][:])
                bb_sb.append(bb)
            bcol_sb = {}
            for i in (3, 4):
                t = const.tile([128, 1], dt.float32, name=f"bcol{i}")
                nc.sync.dma_start(t[:], bcol_ins[i][:])
                bcol_sb[i] = t
            wr_sb = const.tile([128, DIMS[-1]], dt.float32)
            nc.sync.dma_start(wr_sb[:], wr_in[:])
            br_sb = const.tile([128, 1], dt.float32)
            nc.sync.dma_start(br_sb[:], br_in[:])
            ident = const.tile([128, 128], dt.float16)
            make_identity(nc, ident[:])

            def agg_dr(src_tile, fsl_n, planes):
                """Flipped DoubleRow aggregation, one matmul per psum bank:
                returns (ups, u2): ups[fs][gi] = [128, 512] bank (j-groups
                0-3, 4-7), u2 = [128, fsl_n, 256] bank shared by j 8-9."""
                ups = [
                    [upool.tile([128, 512], dt.float32, tag=f"a{gi}_{fs}", name=f"u{gi}{fs}")
                     for gi in range(2)]
                    for fs in range(fsl_n)
                ]
                u2 = upool.tile([128, fsl_n, 256], dt.float32, tag="a2", name="u2")
                n_pl = len(planes)
                for itp in range(PAIRS):
                    for pl in range(n_pl):
                        for fs in range(fsl_n):
                            if planes[pl] == "e4":
                                stat = src_tile[:, 2 * itp:2 * itp + 2, fs * 128:(fs + 1) * 128]
                            else:  # e5 lo plane at byte offset 128
                                stat = src_tile[:, 2 * itp:2 * itp + 2, 128:256].bitcast(dt.float8e5)
                            first = itp == 0 and pl == 0
                            last = itp == PAIRS - 1 and pl == n_pl - 1
                            nc.tensor.matmul(
                                ups[fs][0][:], stat, pt_sb[:, itp, :, 0:4, :],
                                start=first, stop=last,
                                perf_mode=DRM, skip_group_check=True)
                            nc.tensor.matmul(
                                ups[fs][1][:], stat, pt_sb[:, itp, :, 4:8, :],
                                start=first, stop=last,
                                perf_mode=DRM, skip_group_check=True)
                            nc.tensor.matmul(
                                u2[:, fs, :], stat, pt_sb[:, itp, :, 8:10, :],
                                start=(first and fs == 0),
                                stop=(last and fs == fsl_n - 1),
                                perf_mode=DRM, skip_group_check=True)
                return ups, u2

            def uslice(ups, u2, fs, j):
                if j >= 8:
                    return u2[:, fs, (j - 8) * 128:(j - 7) * 128]
                gi, jl = (0, j) if j < 4 else (1, j - 4)
                return ups[fs][gi][:, jl * 128:(jl + 1) * 128]

            def copy_u16(ups, u2, fsl_n):
                u16 = work.tile([128, 2, SHARD_PAD], dt.float16, tag="u16", name="u16")
                for fs in range(fsl_n):
                    nc.vector.tensor_copy(u16[:, fs, 1024:1280], u2[:, fs, :])
                    nc.scalar.activation(u16[:, fs, 0:512], ups[fs][0][:], AF.Copy)
                    nc.vector.tensor_copy(u16[:, fs, 512:1024], ups[fs][1][:])
                return u16

            def stage_chunk(b, oq_tile, view, ch):
                # chain DMAs ride the Activation HWDGE ring so they never
                # queue behind bulk loads/reloads on the SP ring
                w, _ = AG_CFG[b]
                js = JGROUPS[ch]
                nc.sync.dma_start(
                    ag_in[b][ch].rearrange("(j p) c -> p j c", p=128),
                    oq_tile[:, js[0]:js[-1] + 1, 0:w],
                )
                do_ag(b, ch)
                q0 = CH_OFF[ch]
                nb = CH_BLK[ch]
                rb = ag_out[b][ch].rearrange("(r j p) c -> p (r j) c", r=N_CORES, p=128)
                # byte-aware split: deep pipelining for big chunks, but
                # keep sub-DMAs >= ~256KB so fixed per-DMA overhead stays small
                bytes_ = nb * 128 * w * (1 if AG_CFG[b][1] == "e4" else 2)
                parts = 8 if bytes_ >= 786432 else (4 if bytes_ >= 262144 else 2)
                step = nb // parts
                for pi in range(parts):
                    nc.sync.dma_start(view[:, q0 + pi * step:q0 + (pi + 1) * step, 0:w],
                                      rb[:, pi * step:(pi + 1) * step, :])

            def quant_hl(px, oq, j):
                """node-major psum px [128, 128] -> hi|lo fused planes."""
                nc.scalar.activation(oq[:, j, 0:128], px, AF.Copy)
                hi32 = work.tile([128, 128], dt.float32, tag="hi32", name="hi32", bufs=2)
                nc.scalar.activation(hi32[:], oq[:, j, 0:128], AF.Copy)
                nc.vector.tensor_tensor(
                    oq[:, j, 128:256].bitcast(dt.float8e5), px, hi32[:], OP.subtract)

            def make_bd(li):
                """bd_all[:, j] = dinv[j]*b — emitted right after the agg
                matmuls so the DVE computes it while the PE aggregates."""
                dw = DIMS[li + 1]
                bd_all = work.tile([128, JT, 256], dt.float32, tag="bd", name="bd", bufs=1)
                for j in range(JT):
                    nc.vector.tensor_scalar(bd_all[:, j, :dw], bb_sb[li][:], d1[:, j:j + 1],
                                            None, op0=OP.mult)
                return bd_all

            def b_epilogue(li, j, px, out_tile, bd_all):
                """node-major: out[:, j] = relu(dinv2*px + dinv*b)."""
                dw = DIMS[li + 1]
                v = work.tile([128, 256], dt.float32, tag="v", name="v", bufs=2)
                nc.vector.scalar_tensor_tensor(v[:, :dw], px, d2[:, j:j + 1], bd_all[:, j, :dw], OP.mult, OP.add)
                nc.scalar.activation(out_tile, v[:, :dw], AF.Relu)

            def agg_f(srcv, w_in):
                """fp16 direct aggregation, chunk-major src order; j-tile
                accumulators packed into bank tiles a0_0 (j0-7) + a2 (j8-9)."""
                paA = upool.tile([128, 512], dt.float32, tag="a0_0", name="paA")
                paB = upool.tile([128, 1, 256], dt.float32, tag="a2", name="paB")
                for q in range(NB):
                    for j in range(JT):
                        dst = (paA[:, j * 64:(j + 1) * 64] if j < 8
                               else paB[:, 0, (j - 8) * 64:(j - 7) * 64])
                        nc.tensor.matmul(
                            dst[:, 0:w_in] if w_in < 64 else dst,
                            pt_sb[:, q // 2, q % 2, j, :], srcv[:, q, 0:w_in],
                            start=(q == 0 and j in (0, 8)),
                            stop=(q == NB - 1 and j in (7, 9)),
                            skip_group_check=True)
                def pa(j):
                    t = paA[:, j * 64:(j + 1) * 64] if j < 8 else paB[:, 0, (j - 8) * 64:(j - 7) * 64]
                    return t[:, 0:w_in]
                return pa

            for rep in range(repeats):
                if rep > 0:
                    nc.sync.dma_start(S0[:, :, :], g0q_in[:, :, :])

                # ---------------- L0 (B): u = P @ q(g0) hi+lo, w=128 ----------
                ups, u2 = agg_dr(S0, 1, ["e4", "e5"])
                bd_all = make_bd(0)
                u16 = copy_u16(ups, u2, 1)
                oq = work.tile([128, JT, 256], dt.float8e4, tag="oq", name="oq", bufs=2)
                for ch, js in enumerate(JGROUPS):
                    for j in js:
                        px = wpool.tile([128, 256], dt.float32, tag="px", name="px")
                        nc.tensor.matmul(px[:], u16[:, 0, j * 128:(j + 1) * 128], w_sb[0][:, 0, :],
                                         start=True, stop=True)
                        b_epilogue(0, j, px[:], oq[:, j, :], bd_all)
                    stage_chunk(0, oq, S1, ch)

                # ---------------- L1/L2 (B): hi-only, w=256 -------------------
                for li, bnd, src_t, dst_t in ((1, 1, S1, S0), (2, 2, S0, S1)):
                    ups, u2 = agg_dr(src_t, 2, ["e4"])
                    bd_all = make_bd(li)
                    u16 = copy_u16(ups, u2, 2)
                    if li == 1:
                        oq = work.tile([128, JT, 256], dt.float8e4, tag="oq", name="oq", bufs=2)
                        for ch, js in enumerate(JGROUPS):
                            for j in js:
                                px = wpool.tile([128, 256], dt.float32, tag="px", name="px")
                                for fs in range(2):
                                    nc.tensor.matmul(px[:], u16[:, fs, j * 128:(j + 1) * 128],
                                                     w_sb[1][:, fs, :], start=(fs == 0), stop=(fs == 1))
                                b_epilogue(1, j, px[:], oq[:, j, :], bd_all)
                            stage_chunk(bnd, oq, dst_t, ch)
                    else:
                        # L2 + boundary transform to x3 (hi|lo)
                        g3 = work.tile([128, JT, 256], dt.float16, tag="g3", name="g3")
                        g3T = work.tile([128, 2, SHARD_PAD], dt.float16, tag="g3T", name="g3T")
                        oqh = work.tile([128, JT, 256], dt.float8e4, tag="oq", name="oq", bufs=2)
                        for ch, js in enumerate(JGROUPS):
                            for j in js:
                                px = wpool.tile([128, 256], dt.float32, tag="px", name="px")
                                for fs in range(2):
                                    nc.tensor.matmul(px[:], u16[:, fs, j * 128:(j + 1) * 128],
                                                     w_sb[2][:, fs, :], start=(fs == 0), stop=(fs == 1))
                                b_epilogue(2, j, px[:], g3[:, j, :], bd_all)
                                for fb in range(2):
                                    ptr = wpool.tile([128, 128], dt.float16, tag="px", name="ptr")
                                    nc.tensor.transpose(ptr[:], g3[:, j, fb * 128:(fb + 1) * 128], ident[:])
                                    nc.vector.tensor_copy(g3T[:, fb, j * 128:(j + 1) * 128], ptr[:])
                                px3 = wpool.tile([128, 128], dt.float32, tag="px", name="px3")
                                for fb in range(2):
                                    nc.tensor.matmul(px3[:], g3T[:, fb, j * 128:(j + 1) * 128],
                                                     w_sb[3][:, fb, :], start=(fb == 0), stop=(fb == 1))
                                quant_hl(px3[:], oqh, j)
                            stage_chunk(bnd, oqh, dst_t, ch)

                # ---------------- L3/L4 (C): hi+lo, w=128 ---------------------
                for li, bnd, src_t, dst_t in ((3, 3, S1, S0), (4, 4, S0, S1)):
                    ups, u2 = agg_dr(src_t, 1, ["e4", "e5"])
                    gT = work.tile([128, SHARD_PAD], dt.float16, tag="gT", name="gT")
                    if li == 3:
                        oqh = work.tile([128, JT, 256], dt.float8e4, tag="oq", name="oq", bufs=2)
                    else:
                        o16 = work.tile([128, JT, 64], dt.float16, tag="o16", name="o16", bufs=2)
                    for ch, js in enumerate(JGROUPS):
                        for j in js:
                            v = work.tile([128, 256], dt.float32, tag="v", name="v", bufs=2)
                            nc.vector.tensor_tensor(v[:, :128], uslice(ups, u2, 0, j), d2bc[:, j, :], OP.mult)
                            nc.vector.scalar_tensor_tensor(
                                v[:, :128], d1bc[:, j, :], bcol_sb[li][:, 0:1], v[:, :128], OP.mult, OP.add)
                            nc.scalar.activation(gT[:, j * 128:(j + 1) * 128], v[:, :128], AF.Relu)
                            pxn = wpool.tile([128, DIMS[li + 2]], dt.float32, tag="px", name="pxn")
                            nc.tensor.matmul(pxn[:], gT[:, j * 128:(j + 1) * 128], w_sb[li + 1][:, 0, :],
                                             start=True, stop=True)
                            if li == 3:
                                quant_hl(pxn[:], oqh, j)
                            else:
                                nc.scalar.activation(o16[:, j, :], pxn[:], AF.Copy)
                        if li == 3:
                            stage_chunk(bnd, oqh, dst_t, ch)
                        else:
                            stage_chunk(bnd, o16, dst_t.bitcast(dt.float16), ch)

                # ---------------- L5/L6 (F): fp16 direct agg ------------------
                for li, bnd, src_t, dst_t in ((5, 5, S1, S0), (6, 6, S0, S1)):
                    srcv = src_t.bitcast(dt.float16)
                    dstv = dst_t.bitcast(dt.float16)
                    w_out = DIMS[li + 2]
                    pa = agg_f(srcv, 64)
                    bd_all = make_bd(li)
                    gn = work.tile([128, JT, 64], dt.float16, tag="gn", name="gn")
                    gnT = work.tile([64, SHARD_PAD], dt.float16, tag="gnT", name="gnT")
                    o16 = work.tile([128, JT, 64], dt.float16, tag="o16", name="o16", bufs=2)
                    for ch, js in enumerate(JGROUPS):
                        for j in js:
                            b_epilogue(li, j, pa(j), gn[:, j, :], bd_all)
                            ptr = wpool.tile([128, 128], dt.float16, tag="px", name="ptr")
                            nc.tensor.transpose(ptr[:64, :], gn[:, j, :], ident[:])
                            nc.vector.tensor_copy(gnT[:, j * 128:(j + 1) * 128], ptr[:64, :])
                            pxn = wpool.tile([128, w_out], dt.float32, tag="px", name="pxn")
                            nc.tensor.matmul(pxn[:], gnT[:, j * 128:(j + 1) * 128],
                                             w_sb[li + 1][0:64, 0, :], start=True, stop=True)
                            nc.scalar.activation(o16[:, j, 0:w_out], pxn[:], AF.Copy)
                        stage_chunk(bnd, o16, dstv, ch)

                # ---------------- L7 (F): w=32 agg + readout ------------------
                srcv = S1.bitcast(dt.float16)
                pa = agg_f(srcv, 32)
                red_all = work.tile([128, JT], dt.float32, tag="red", name="red", bufs=2)
                for j in range(JT):
                    v = work.tile([128, 256], dt.float32, tag="v", name="v", bufs=2)
                    nc.vector.scalar_tensor_tensor(
                        v[:, :32], pa(j), d1[:, j:j + 1], bb_sb[7][:], OP.mult, OP.add)
                    h8 = work.tile([128, 32], dt.float32, tag="h8", name="h8", bufs=2)
                    nc.scalar.activation(h8[:], v[:, :32], AF.Relu)
                    prod = work.tile([128, 32], dt.float32, tag="prod", name="prod", bufs=2)
                    nc.vector.scalar_tensor_tensor(
                        prod[:], h8[:], 1.0, wr_sb[:], OP.mult, OP.mult,
                        accum_out=red_all[:, j:j + 1])
                    nc.vector.tensor_scalar(red_all[:, j:j + 1], red_all[:, j:j + 1],
                                            br_sb[:, 0:1], None, op0=OP.add)
                nc.sync.dma_start(out_dram.rearrange("j p o -> p (j o)"), red_all[:])

    nc.compile()
    return nc


_NC_CACHE = None


def _get_nc():
    global _NC_CACHE
    if _NC_CACHE is None:
        _NC_CACHE = _build_bass()
    return _NC_CACHE


def _pad_index(g):
    k = g // SHARD
    return k * SHARD_PAD + (g - k * SHARD)


def _prepare_inputs(inputs):
    x = np.asarray(inputs["x"], np.float32)
    ei = np.asarray(inputs["edge_index"])
    src, dst = ei[0].astype(np.int64), ei[1].astype(np.int64)

    deg = np.zeros(N_NODES, np.float32)
    np.add.at(deg, dst, 1.0)
    deg += 1.0
    dinv = 1.0 / np.sqrt(deg)

    psrc = _pad_index(src)
    pdst = _pad_index(dst)
    pself = _pad_index(np.arange(N_NODES, dtype=np.int64))

    P = np.zeros((NPAD, NPAD), np.float32)
    np.add.at(P, (pdst, psrc), 1.0)
    P[pself, pself] += 1.0
    assert P.max() <= 15

    dinv_pad = np.zeros(NPAD, np.float32)
    dinv_pad[pself] = dinv

    sigma = _sigma()

    g0 = np.zeros((NPAD, 128), np.float32)
    g0[pself] = dinv[:, None] * x
    g0b = g0.reshape(NB, 128, 128)[sigma]          # [q, p, f]
    g0hi = g0b.astype(E4)
    g0lo = (g0b - g0hi.astype(np.float32)).astype(E5)
    g0q = np.concatenate([g0hi, g0lo.view(E4)], axis=2)   # [q, p, 256]
    g0q = np.ascontiguousarray(g0q.transpose(1, 0, 2))    # [p, q, 256]

    FT = [max(1, DIMS[i] // 128) for i in range(8)]
    w_np = []
    for li in range(8):
        W = np.asarray(inputs[f"W{li}"], np.float32)
        Wp = np.zeros((FT[li] * 128, DIMS[li + 1]), np.float32)
        Wp[: W.shape[0]] = W
        w_np.append(np.ascontiguousarray(
            Wp.reshape(FT[li], 128, DIMS[li + 1]).transpose(1, 0, 2)).astype(F16))
    bb_np = [
        np.broadcast_to(np.asarray(inputs[f"b{li}"], np.float32), (128, DIMS[li + 1])).copy()
        for li in range(8)
    ]
    bcol_np = {}
    for i in (3, 4):
        b = np.asarray(inputs[f"b{i}"], np.float32)
        col = np.zeros((128, 1), np.float32)
        col[: b.shape[0], 0] = b
        bcol_np[i] = col
    wr = np.asarray(inputs["Wr"], np.float32)
    wr_np = np.zeros((128, DIMS[-1]), np.float32)
    wr_np[:, :] = wr[:, 0][None, :]
    br_np = np.full((128, 1), np.asarray(inputs["br"], np.float32).reshape(()), np.float32)

    in_maps = []
    for k in range(N_CORES):
        rows = slice(k * SHARD_PAD, (k + 1) * SHARD_PAD)
        S = P[rows].reshape(JT, 128, NB, 128)      # [jt, c, blk, p]
        S = S[:, :, sigma, :]
        # pt[p, itp, pair, jt, c] = P[own_row(jt, c), src_block(2*itp+pair)*128 + p]
        pt = np.ascontiguousarray(S.transpose(3, 2, 0, 1)).astype(E4)  # [p, q, jt, c]
        pt = pt.reshape(128, PAIRS, 2, JT, 128)
        dj = dinv_pad[rows].reshape(JT, 128)       # [jt, c]
        m = {
            "pt_in": pt,
            "g0q_in": g0q,
            "d1_in": np.ascontiguousarray(dj.T),
            "d2_in": np.ascontiguousarray((dj * dj).T),
            "d1bc_in": np.broadcast_to(dj[None, :, :], (128, JT, 128)).copy(),
            "d2bc_in": np.broadcast_to((dj * dj)[None, :, :], (128, JT, 128)).copy(),
            "wr_in": wr_np,
            "br_in": br_np,
            "bcol3_in": bcol_np[3],
            "bcol4_in": bcol_np[4],
        }
        for li in range(8):
            m[f"w{li}_in"] = w_np[li]
            m[f"bb{li}_in"] = bb_np[li]
        in_maps.append(m)
    return in_maps


def kernel(**inputs):
    nc = _get_nc()
    in_maps = _prepare_inputs(inputs)
    res = bass_utils.run_bass_kernel_spmd(nc, in_maps, core_ids=list(range(N_CORES)))
    out = np.empty((N_NODES, D_OUT), np.float32)
    for k in range(N_CORES):
        shard = res.results[k]["out"].reshape(SHARD_PAD, D_OUT)
        out[k * SHARD:(k + 1) * SHARD] = shard[:SHARD]
    return out
